# revision 6
# baseline (speedup 1.0000x reference)
"""M-trick variant: out = (A^T vw) @ x + (A^T vb) 1^T, eliminating the
v-conv GEMM entirely.

Math per batch (C=512, N=4096):
  G = x @ qT                  [C,C]   (fp16 operands, fp32 PSUM accum)
  E = kw @ G + kb (.) S       [C,C]   S = colsum(qT), computed on host
  A = gamma * softmax(E)      rows normalized on-chip
  M^T = vw^T @ A              [C,C]   tiny GEMM replaces the v-conv
  bias = A^T vb               [C]
  out_attn = M^T.T @ x + bias [C,N]   fp16 operands
Host adds the +x residual and upcasts the fp16 out_attn to fp32.

vs the G-trick baseline this removes the 64K-row v-conv and the 16K-row
on-chip S computation (~226K -> ~150K PE rows), and the fp16 streams halve
DMA bytes (logit-path fp16 keeps softmax-amplified quantization ~4x below
bf16; measured rel_absmax 2.3e-3 vs the 2e-2 gate).  DMAs are packed into
[128, 2048]-class transfers because every DMA serializes ~625ns on the
global HWDGE resource.
"""

import numpy as np
import ml_dtypes

import concourse.bass as bass
import concourse.tile as tile
from concourse import bacc, mybir
from concourse.bass_utils import run_bass_kernel_spmd

F32 = mybir.dt.float32
F32R = mybir.dt.float32r
F16 = mybir.dt.float16
AX = mybir.AxisListType
AF = mybir.ActivationFunctionType
ALU = mybir.AluOpType

C = 512
N = 4096
P = 128
CT = C // P          # 4 chunks of C
NS = N // P          # 32 n-chunks of 128
NQ = NS // 4         # 8 quad n-chunks (one [128, 2048] DMA each)
NCH = N // 512       # 8 column panels of 512
NCORES = 8

_cached = {}


def _build_program(repeat=1):
    from contextlib import ExitStack

    nc = bacc.Bacc("TRN2", target_bir_lowering=False, debug=False,
                   num_devices=NCORES)

    # packed layouts (see make_in_maps): each DMA is a contiguous row-slice
    xtq_d = nc.dram_tensor("xtq", [NQ * P, 4 * C], F16, kind="ExternalInput").ap()
    qtq_d = nc.dram_tensor("qtq", [NQ * P, 4 * C], F16, kind="ExternalInput").ap()
    x16_d = nc.dram_tensor("x16", [C, N], F16, kind="ExternalInput").ap()
    kwT_d = nc.dram_tensor("kwT", [P, CT * C], F32, kind="ExternalInput").ap()
    vw_d = nc.dram_tensor("vw", [P, CT * C], F32, kind="ExternalInput").ap()
    krs_d = nc.dram_tensor("krs", [1, 2 * C], F32, kind="ExternalInput").ap()
    # vb chunks interleaved with zero columns (fp32r matmuls need even free
    # dims), gamma in the last column: [vb0, 0, vb1, 0, vb2, 0, vb3, 0, gam]
    vbg_d = nc.dram_tensor("vbg", [P, 2 * CT + 1], F32, kind="ExternalInput").ap()
    out_d = nc.dram_tensor("out", [C, N], F16, kind="ExternalOutput").ap()

    with tile.TileContext(nc) as tc:
        with (
            tc.tile_pool(name="big", bufs=1) as big,
            tc.tile_pool(name="xtp", bufs=3) as xtp,
            tc.tile_pool(name="qtp", bufs=3) as qtp,
            tc.tile_pool(name="otp", bufs=3) as otp,
            tc.tile_pool(name="small", bufs=1) as small,
        ):
            for _rep in range(repeat):
                x16_sb = [big.tile([P, N], F16, tag=f"x{i}", name=f"x{i}")
                          for i in range(CT)]
                kwT_sb = big.tile([P, CT * C], F32R, tag="kw", name="kw")
                vw_sb = big.tile([P, CT * C], F32R, tag="vw", name="vw")
                G_sb = [big.tile([P, C], F32R, tag=f"g{i}", name=f"g{i}")
                        for i in range(CT)]
                a_sb = [big.tile([P, C], F32R, tag=f"a{i}", name=f"a{i}")
                        for i in range(CT)]
                mgT_sb = [big.tile([P, C], F16, tag=f"m{i}", name=f"m{i}")
                          for i in range(CT)]

                vbg_sb = small.tile([P, 2 * CT + 1], F32R, tag="vbg")
                nc.sync.dma_start(vbg_sb, vbg_d[:].bitcast(F32R))

                gps_stack = ExitStack()
                gps = gps_stack.enter_context(
                    tc.tile_pool(name="gps", bufs=1, space="PSUM"))
                g_ps = [gps.tile([P, C], F32, tag=f"gp{i}", name=f"gp{i}")
                        for i in range(CT)]

                # ---- phase 1: G accumulation over 8 quad-chunks ----
                for q in range(NQ):
                    xt = xtp.tile([P, 4 * C], F16, tag="xt", name="xt")
                    nc.sync.dma_start(xt, xtq_d[q * P:(q + 1) * P, :])
                    qt = qtp.tile([P, 4 * C], F16, tag="qt", name="qt")
                    nc.sync.dma_start(qt, qtq_d[q * P:(q + 1) * P, :])
                    if q == 2:
                        nc.sync.dma_start(kwT_sb, kwT_d[:].bitcast(F32R))
                        krs_sb = small.tile([1, 2 * C], F32R, tag="krs",
                                            name="krs")
                        nc.sync.dma_start(krs_sb, krs_d[:].bitcast(F32R))
                    if q == 3:
                        nc.sync.dma_start(vw_sb, vw_d[:].bitcast(F32R))
                    if q >= 4:
                        i = q - 4
                        nc.sync.dma_start(x16_sb[i],
                                          x16_d[i * P:(i + 1) * P, :])
                    for k in range(4):
                        ns = q * 4 + k
                        for ct in range(CT):
                            nc.tensor.matmul(
                                g_ps[ct][:],
                                xt[:, k * C + ct * P:k * C + (ct + 1) * P],
                                qt[:, k * C:(k + 1) * C],
                                start=(ns == 0), stop=(ns == NS - 1))

                # ---- evict G to SBUF (alternate DVE/ACT) ----
                for ct in range(CT):
                    if ct % 2 == 0:
                        nc.vector.tensor_copy(G_sb[ct][:], g_ps[ct][:])
                    else:
                        nc.scalar.activation(G_sb[ct][:], g_ps[ct][:], AF.Copy)

                gps_stack.close()
                mps_stack = ExitStack()
                mps = mps_stack.enter_context(
                    tc.tile_pool(name="mps", bufs=1, space="PSUM"))
                eps_stack = ExitStack()
                eps = eps_stack.enter_context(
                    tc.tile_pool(name="eps", bufs=1, space="PSUM"))
                mT_ps = [mps.tile([P, C], F32, tag=f"mp{j}", name=f"mp{j}")
                         for j in range(CT)]
                b_ps = mps.tile([P, C], F32, tag="bp", name="b_ps")

                # ---- E = kw @ G + kb (.) S ; softmax ; MT/bias interleave ----
                nmx = [small.tile([P, 1], F32, tag=f"nmx{i}", name=f"nmx{i}")
                       for i in range(CT)]
                ssum = [small.tile([P, 1], F32, tag=f"ss{i}", name=f"ss{i}")
                        for i in range(CT)]
                rs = [small.tile([P, 1], F32, tag=f"rs{i}", name=f"rs{i}")
                      for i in range(CT)]
                for i in range(CT):
                    e_ps = eps.tile([P, C], F32, tag="e", name="e_ps", bufs=2)
                    for ct in range(CT):
                        nc.tensor.matmul(
                            e_ps[:],
                            kwT_sb[:, ct * C + i * P:ct * C + (i + 1) * P],
                            G_sb[ct][:], start=(ct == 0), stop=False)
                    nc.tensor.matmul(e_ps[:], krs_sb[0:1, i * P:(i + 1) * P],
                                     krs_sb[0:1, C:2 * C], start=False,
                                     stop=True)
                    nc.vector.reduce_max(nmx[i][:], e_ps[:], axis=AX.X,
                                         negate=True)
                    nc.scalar.activation(a_sb[i][:], e_ps[:], AF.Exp,
                                         bias=nmx[i][:, 0:1], scale=1.0,
                                         accum_out=ssum[i][:, 0:1])
                    nc.vector.reciprocal(rs[i][:], ssum[i][:])
                    # a = exp(.) * (gamma / rowsum), per-partition scalars
                    nc.vector.tensor_scalar(
                        out=a_sb[i][:], in0=a_sb[i][:],
                        scalar1=rs[i][:, 0:1],
                        scalar2=vbg_sb[:, 2 * CT:2 * CT + 1].bitcast(F32),
                        op0=ALU.mult, op1=ALU.mult)
                    # MT accumulation: mT_ps[j] += vw[d in i, e in j]^T @ a[i]
                    for j in range(CT):
                        nc.tensor.matmul(
                            mT_ps[j][:],
                            vw_sb[:, i * C + j * P:i * C + (j + 1) * P],
                            a_sb[i][:], start=(i == 0), stop=(i == CT - 1))
                    # bias cols (paired with a zero col for even free):
                    # b_ps[:, 2j] += a[i][:, j-slice]^T @ vb[i]
                    for j in range(CT):
                        nc.tensor.matmul(
                            b_ps[:, 2 * j:2 * j + 2],
                            a_sb[i][:, j * P:(j + 1) * P],
                            vbg_sb[:, 2 * i:2 * i + 2], start=(i == 0),
                            stop=(i == CT - 1))

                eps_stack.close()

                # ---- evict MT (fp16) and bias columns ----
                for j in range(CT):
                    if j % 2 == 0:
                        nc.vector.tensor_copy(mgT_sb[j][:], mT_ps[j][:])
                    else:
                        nc.scalar.activation(mgT_sb[j][:], mT_ps[j][:],
                                             AF.Copy)
                bias_sb = small.tile([P, 2 * CT], F32, tag="bias",
                                     name="bias_sb")
                nc.vector.tensor_copy(bias_sb[:], b_ps[:, 0:2 * CT])

                mps_stack.close()
                ops_stack = ExitStack()
                ops = ops_stack.enter_context(
                    tc.tile_pool(name="ops", bufs=1, space="PSUM"))

                # ---- out = MgT^T @ x + bias ----
                for j in range(CT):
                    for half in range(2):
                        o_ps = ops.tile([P, 4 * C], F32, tag="o", name="o_ps",
                                        bufs=2)
                        for chs in range(4):
                            ch = half * 4 + chs
                            for i in range(CT):
                                nc.tensor.matmul(
                                    o_ps[:, chs * C:(chs + 1) * C],
                                    mgT_sb[i][:, j * P:(j + 1) * P],
                                    x16_sb[i][:, ch * C:(ch + 1) * C],
                                    start=(i == 0), stop=(i == CT - 1))
                        ot = otp.tile([P, 4 * C], F16, tag="ot", name="ot")
                        nc.vector.tensor_scalar_add(ot[:], o_ps[:],
                                                    bias_sb[:, 2 * j:2 * j + 1])
                        nc.sync.dma_start(
                            out_d[j * P:(j + 1) * P,
                                  half * 4 * C:(half + 1) * 4 * C], ot[:])
                ops_stack.close()

    nc.compile()
    return nc


def _get_program(repeat=1):
    if repeat not in _cached:
        _cached[repeat] = _build_program(repeat)
    return _cached[repeat]


def make_in_maps(x, proj_query, key_w, key_b, value_w, value_b, gamma):
    """Per-core input dicts: batch-parallel shards + replicated weights."""
    B = x.shape[0]
    xb = np.asarray(x, dtype=np.float32).reshape(B, C, N)
    qm = np.asarray(proj_query, dtype=np.float32).reshape(C, N)

    def pack_quads(aT16):
        # [N, C] -> [NQ*P, 4*C] where row q*P+p holds chunks (q*4+k)*P+p
        return np.ascontiguousarray(
            aT16.reshape(NQ, 4, P, C).transpose(0, 2, 1, 3)
            .reshape(NQ * P, 4 * C))

    qT16 = qm.T.astype(np.float16)
    qtq = pack_quads(qT16)

    def pack_rows(w):
        # [C, C] -> [P, CT*C]: row p holds [w[p], w[P+p], w[2P+p], w[3P+p]]
        return np.ascontiguousarray(
            w.reshape(CT, P, C).transpose(1, 0, 2).reshape(P, CT * C)
            .astype(np.float32))

    kwT = pack_rows(np.asarray(key_w, dtype=np.float32).T)
    vw = pack_rows(np.asarray(value_w, dtype=np.float32))

    S = qm.astype(np.float64).sum(axis=1).astype(np.float32)
    krs = np.ascontiguousarray(
        np.concatenate([np.asarray(key_b, np.float32), S])[None, :])
    vbc = np.asarray(value_b, np.float32).reshape(CT, P).T  # [P, CT]
    vbg = np.zeros((P, 2 * CT + 1), np.float32)
    vbg[:, 0:2 * CT:2] = vbc
    vbg[:, 2 * CT] = np.asarray(gamma, np.float32).reshape(1)[0]

    maps = []
    for b in range(B):
        maps.append({
            "xtq": pack_quads(xb[b].T.astype(np.float16)),
            "qtq": qtq,
            "x16": np.ascontiguousarray(xb[b].astype(np.float16)),
            "kwT": kwT, "vw": vw, "krs": krs, "vbg": vbg,
        })
    return maps


def kernel(x, proj_query, key_w, key_b, value_w, value_b, gamma, **_unused):
    B, Cx, W, H = x.shape
    assert (B, Cx, W * H) == (NCORES, C, N)
    nc = _get_program()
    in_maps = make_in_maps(x, proj_query, key_w, key_b, value_w, value_b,
                           gamma)
    res = run_bass_kernel_spmd(nc, in_maps, list(range(NCORES)))
    xb = np.asarray(x, dtype=np.float32).reshape(B, C, N)
    out = np.stack([
        res.results[b]["out"].astype(np.float32) + xb[b] for b in range(B)
    ])
    return out.reshape(B, C, W, H).astype(np.float32)
